# revision 4
# baseline (speedup 1.0000x reference)
"""Multi-head attention (B=4, H=16, S=2048, D=128, causal+pad mask) on 8 TRN2 NeuronCores.

Sharding: the 64 (batch, head) pairs are split 8 per core (pure data parallel —
attention is independent per head, no collectives needed).

Per-core kernel (per head):
  - scores are computed TRANSPOSED: S^T[k, q] = (K_block)^T^T @ Q^T with the
    contraction dim d=128 on partitions, k-block (128) as the PSUM partition dim
    and a 256-wide q tile as the moving dim. Inputs are fp32 bitcast to float32r,
    which runs at full PE rate (1 cycle/row) for moving dim >= 256.
  - exp(scale*s) runs on the scalar engine straight out of PSUM into SBUF as
    bf16 (P^T layout). No max-subtraction: scores*scale ~ N(0,1), exp is safe.
  - Partially-masked blocks are zeroed by a bf16 multiply with host-derived mask
    tiles (deduped). Fully-masked blocks are skipped; fully-allowed untouched.
  - P^T lands exactly in the layout the PV matmul needs (k on partitions):
    O[q_sub 128, 132] += P^T[:, sub]^T @ V'[k_block] accumulated over k blocks
    in PSUM, where V' is V in bf16 with a ones column appended at col 128 —
    so O[:, 128] is the softmax denominator for free.
  - reciprocal + per-partition scale normalizes, then DMA out as f32.
"""

import os
import sys

import numpy as np

try:  # the repo root that provides `concourse` / `gauge` / `antenv`
    import concourse.bass  # noqa: F401
except ImportError:  # pragma: no cover
    for _p in ("/opt/trn_rl_repo", "/root/.axon_site/_ro/trn_rl_repo"):
        if os.path.isdir(_p) and _p not in sys.path:
            sys.path.insert(0, _p)

import ml_dtypes

B, H, S, D = 4, 16, 2048, 128
BH = B * H
NCORES = 8
HPC = BH // NCORES  # heads per core = 8
QM = 256  # q tile width (matmul moving dim; >=256 keeps float32r at full rate)
KB = 128  # k block (PSUM partition dim of S^T)
NM = S // QM  # 8 q tiles
NKB = S // KB  # 16 k blocks
VW = D + 4  # V' width: col D holds ones (softmax denom), cols D+1.. are zero pad
SCALE = float(np.float32(1.0 / np.sqrt(np.float32(D))))

_CACHE: dict = {}
LAST_RESULTS = None  # BassKernelResults of the most recent run (for test harness)


def _derive_schedule(attn_mask):
    """Classify each (q-tile, k-block) pair from the actual mask.

    Returns (schedule, mask_tiles) where schedule[m] is a list of
    (j, mask_idx_or_None) for blocks with any allowed element, and mask_tiles is
    the deduped [128, n, QM] bf16 stack of transposed partial-block masks.
    """
    am = np.asarray(attn_mask) != 0  # [S(q), S(k)] bool
    uniq: dict = {}
    tiles = []
    schedule = []
    for m in range(NM):
        row = []
        for j in range(NKB):
            blk = am[m * QM : (m + 1) * QM, j * KB : (j + 1) * KB]  # [QM, KB]
            if not blk.any():
                continue
            if blk.all():
                row.append((j, None))
                continue
            key = blk.tobytes()
            if key not in uniq:
                uniq[key] = len(tiles)
                tiles.append(blk.T.astype(ml_dtypes.bfloat16))  # [KB, QM]
            row.append((j, uniq[key]))
        schedule.append(row)
    mask_tiles = np.stack(tiles, axis=1) if tiles else None  # [128, n, QM]
    return schedule, mask_tiles


def _build_program(schedule, n_masks, use_pad):
    import concourse.mybir as mybir
    import concourse.tile as tile
    from concourse import bacc

    f32 = mybir.dt.float32
    f32r = mybir.dt.float32r
    bf16 = mybir.dt.bfloat16
    Exp = mybir.ActivationFunctionType.Exp

    nc = bacc.Bacc(None)
    qt_ext = nc.declare_dram_parameter("qt", [HPC, 128, S], bf16, isOutput=False)
    kt_ext = nc.declare_dram_parameter("kt", [HPC, 128, S], bf16, isOutput=False)
    vp_ext = nc.declare_dram_parameter("vp", [HPC, 128, NKB, VW], bf16, isOutput=False)
    if n_masks:
        mk_ext = nc.declare_dram_parameter("mk", [128, n_masks, QM], bf16, isOutput=False)
    if use_pad:
        pc_ext = nc.declare_dram_parameter("pc", [128, NKB], bf16, isOutput=False)
    out_ext = nc.declare_dram_parameter("out", [HPC, S, D], f32, isOutput=True)

    with tile.TileContext(nc) as tc:
        with (
            tc.tile_pool(name="qt", bufs=2) as qt_pool,
            tc.tile_pool(name="kt", bufs=2) as kt_pool,
            tc.tile_pool(name="vp", bufs=2) as vp_pool,
            tc.tile_pool(name="pt", bufs=4) as pt_pool,
            tc.tile_pool(name="osb", bufs=3) as osb_pool,
            tc.tile_pool(name="rec", bufs=3) as rec_pool,
            tc.tile_pool(name="mk", bufs=1) as mk_pool,
            tc.tile_pool(name="st", bufs=3, space="PSUM") as st_pool,
            tc.tile_pool(name="ops", bufs=4, space="PSUM") as o_pool,
        ):
            if n_masks:
                mk = mk_pool.tile([128, n_masks, QM], bf16)
                nc.sync.dma_start(mk[:], mk_ext[:])
            if use_pad:
                pc = mk_pool.tile([128, NKB], bf16)
                nc.sync.dma_start(pc[:], pc_ext[:])

            for h in range(HPC):
                qt = qt_pool.tile([128, S], bf16)
                nc.sync.dma_start(qt[:], qt_ext[h])
                kt = kt_pool.tile([128, S], bf16)
                nc.sync.dma_start(kt[:], kt_ext[h])
                vp = vp_pool.tile([128, NKB, VW], bf16)
                nc.sync.dma_start(vp[:], vp_ext[h])

                for m in range(NM):
                    blocks = schedule[m]
                    if not blocks:
                        continue
                    o0 = o_pool.tile([128, VW], f32, tag="o")
                    o1 = o_pool.tile([128, VW], f32, tag="o")
                    n_blk = len(blocks)
                    for idx, (j, mi) in enumerate(blocks):
                        st = st_pool.tile([128, QM], f32)
                        nc.tensor.matmul(
                            st[:],
                            lhsT=kt[:, j * KB : (j + 1) * KB],
                            rhs=qt[:, m * QM : (m + 1) * QM],
                            start=True,
                            stop=True,
                        )
                        pt = pt_pool.tile([128, QM], bf16)
                        nc.scalar.activation(pt[:], st[:], Exp, scale=SCALE)
                        if mi is not None:
                            nc.vector.tensor_mul(pt[:], pt[:], mk[:, mi, :])
                        if use_pad:
                            nc.vector.tensor_scalar_mul(pt[:], pt[:], pc[:, j : j + 1])
                        first = idx == 0
                        last = idx == n_blk - 1
                        nc.tensor.matmul(
                            o0[:], lhsT=pt[:, 0:128], rhs=vp[:, j, :],
                            start=first, stop=last,
                        )
                        nc.tensor.matmul(
                            o1[:], lhsT=pt[:, 128:256], rhs=vp[:, j, :],
                            start=first, stop=last,
                        )
                    for s, o in enumerate((o0, o1)):
                        rec = rec_pool.tile([128, 1], f32)
                        nc.vector.reciprocal(rec[:], o[:, D : D + 1])
                        osb = osb_pool.tile([128, D], f32)
                        nc.vector.tensor_scalar_mul(osb[:], o[:, 0:D], rec[:])
                        row0 = m * QM + s * 128
                        nc.sync.dma_start(out_ext[h, row0 : row0 + 128, :], osb[:])
    nc.compile()
    return nc


def _prep_inputs(q, k, v, attn_mask, pad_mask):
    q = np.asarray(q, dtype=np.float32).reshape(BH, S, D)
    k = np.asarray(k, dtype=np.float32).reshape(BH, S, D)
    v = np.asarray(v, dtype=np.float32).reshape(BH, S, D)

    qt = np.ascontiguousarray(q.transpose(0, 2, 1)).astype(ml_dtypes.bfloat16)
    kt = np.ascontiguousarray(k.transpose(0, 2, 1)).astype(ml_dtypes.bfloat16)

    # V': [BH, 128(row within k block), NKB, VW] bf16; col D = 1.0 (denominator)
    vp = np.zeros((BH, 128, NKB, VW), dtype=ml_dtypes.bfloat16)
    vblocks = v.reshape(BH, NKB, 128, D).transpose(0, 2, 1, 3)
    vp[:, :, :, :D] = vblocks.astype(ml_dtypes.bfloat16)
    vp[:, :, :, D] = 1.0

    pad = np.asarray(pad_mask).reshape(B, S) != 0
    use_pad = not bool(pad.all())
    pcs = None
    if use_pad:
        pcs = []
        for c in range(NCORES):
            b = (c * HPC) // H
            pcs.append(
                np.ascontiguousarray(
                    pad[b].reshape(NKB, 128).T.astype(ml_dtypes.bfloat16)
                )
            )
    return qt, kt, vp, use_pad, pcs


def kernel(q, k, v, attn_mask, pad_mask):
    global LAST_RESULTS
    from concourse.bass_utils import run_bass_kernel_spmd

    schedule, mask_tiles = _derive_schedule(attn_mask)
    qt, kt, vp, use_pad, pcs = _prep_inputs(q, k, v, attn_mask, pad_mask)
    n_masks = 0 if mask_tiles is None else mask_tiles.shape[1]

    key = (
        np.asarray(attn_mask).tobytes(),
        use_pad,
    )
    nc = _CACHE.get(key)
    if nc is None:
        nc = _build_program(schedule, n_masks, use_pad)
        _CACHE[key] = nc

    in_maps = []
    for c in range(NCORES):
        sl = slice(c * HPC, (c + 1) * HPC)
        m = {"qt": qt[sl], "kt": kt[sl], "vp": vp[sl]}
        if n_masks:
            m["mk"] = mask_tiles
        if use_pad:
            m["pc"] = pcs[c]
        in_maps.append(m)

    res = run_bass_kernel_spmd(nc, in_maps, core_ids=list(range(NCORES)))
    LAST_RESULTS = res
    out = np.concatenate([res.results[c]["out"] for c in range(NCORES)], axis=0)
    return np.ascontiguousarray(out.reshape(B, H, S, D).astype(np.float32))


# revision 8
# speedup vs baseline: 1.0195x; 1.0195x over previous
"""Multi-head attention (B=4, H=16, S=2048, D=128, causal+pad mask) on 8 TRN2 NeuronCores.

Sharding: the 64 (batch, head) pairs are split 8 per core (pure data parallel —
attention is independent per head, no collectives needed).

Per-core kernel (per head):
  - scores are computed TRANSPOSED: S^T[k, q] = (K_block)^T^T @ Q^T with the
    contraction dim d=128 on partitions, k-block (128) as the PSUM partition dim
    and a 256-wide q tile as the moving dim. Inputs are fp32 bitcast to float32r,
    which runs at full PE rate (1 cycle/row) for moving dim >= 256.
  - exp(scale*s) runs on the scalar engine straight out of PSUM into SBUF as
    bf16 (P^T layout). No max-subtraction: scores*scale ~ N(0,1), exp is safe.
  - Partially-masked blocks are zeroed by a bf16 multiply with host-derived mask
    tiles (deduped). Fully-masked blocks are skipped; fully-allowed untouched.
  - P^T lands exactly in the layout the PV matmul needs (k on partitions):
    O[q_sub 128, 132] += P^T[:, sub]^T @ V'[k_block] accumulated over k blocks
    in PSUM, where V' is V in bf16 with a ones column appended at col 128 —
    so O[:, 128] is the softmax denominator for free.
  - reciprocal + per-partition scale normalizes, then DMA out as f32.
"""

import os
import sys

import numpy as np

try:  # the repo root that provides `concourse` / `gauge` / `antenv`
    import concourse.bass  # noqa: F401
except ImportError:  # pragma: no cover
    for _p in ("/opt/trn_rl_repo", "/root/.axon_site/_ro/trn_rl_repo"):
        if os.path.isdir(_p) and _p not in sys.path:
            sys.path.insert(0, _p)

import ml_dtypes

B, H, S, D = 4, 16, 2048, 128
BH = B * H
NCORES = 8
HPC = BH // NCORES  # heads per core = 8
QM = 256  # q tile width (matmul moving dim; >=256 keeps float32r at full rate)
KB = 128  # k block (PSUM partition dim of S^T)
NM = S // QM  # 8 q tiles
NKB = S // KB  # 16 k blocks
VW = D + 4  # V' width: col D holds ones (softmax denom), cols D+1.. are zero pad
SCALE = float(np.float32(1.0 / np.sqrt(np.float32(D))))

_CACHE: dict = {}
LAST_RESULTS = None  # BassKernelResults of the most recent run (for test harness)


def _derive_schedule(attn_mask):
    """Classify each (q-tile, k-block) pair from the actual mask.

    Returns (schedule, mask_tiles) where schedule[m] is a list of
    (j, mask_idx_or_None) for blocks with any allowed element, and mask_tiles is
    the deduped [128, n, QM] bf16 stack of transposed partial-block masks.
    """
    am = np.asarray(attn_mask) != 0  # [S(q), S(k)] bool
    uniq: dict = {}
    tiles = []
    schedule = []
    for m in range(NM):
        row = []
        for j in range(NKB):
            blk = am[m * QM : (m + 1) * QM, j * KB : (j + 1) * KB]  # [QM, KB]
            if not blk.any():
                continue
            if blk.all():
                row.append((j, None))
                continue
            key = blk.tobytes()
            if key not in uniq:
                uniq[key] = len(tiles)
                tiles.append(blk.T.astype(ml_dtypes.bfloat16))  # [KB, QM]
            row.append((j, uniq[key]))
        schedule.append(row)
    mask_tiles = np.stack(tiles, axis=1) if tiles else None  # [128, n, QM]
    return schedule, mask_tiles


def _build_program(schedule, n_masks, use_pad):
    import concourse.mybir as mybir
    import concourse.tile as tile
    from concourse import bacc

    f32 = mybir.dt.float32
    f32r = mybir.dt.float32r
    bf16 = mybir.dt.bfloat16
    Exp = mybir.ActivationFunctionType.Exp

    nc = bacc.Bacc(None)
    qt_ext = nc.declare_dram_parameter("qt", [HPC, 128, S], bf16, isOutput=False)
    kt_ext = nc.declare_dram_parameter("kt", [HPC, 128, S], bf16, isOutput=False)
    vp_ext = nc.declare_dram_parameter("vp", [HPC, 128, NKB, VW], bf16, isOutput=False)
    if n_masks:
        mk_ext = nc.declare_dram_parameter("mk", [128, n_masks, QM], bf16, isOutput=False)
    if use_pad:
        pc_ext = nc.declare_dram_parameter("pc", [128, NKB], bf16, isOutput=False)
    out_ext = nc.declare_dram_parameter("out", [HPC, S, D], f32, isOutput=True)

    G = 6  # k-blocks per exp group: st group [128, G*QM] f32 = 3 PSUM banks

    with tile.TileContext(nc) as tc:
        with (
            tc.tile_pool(name="qt", bufs=2) as qt_pool,
            tc.tile_pool(name="kt", bufs=2) as kt_pool,
            tc.tile_pool(name="vp", bufs=2) as vp_pool,
            tc.tile_pool(name="pt", bufs=2) as pt_pool,
            tc.tile_pool(name="osb", bufs=3) as osb_pool,
            tc.tile_pool(name="rec", bufs=3) as rec_pool,
            tc.tile_pool(name="mk", bufs=1) as mk_pool,
            tc.tile_pool(name="st", bufs=2, space="PSUM") as st_pool,
            tc.tile_pool(name="ops", bufs=2, space="PSUM") as o_pool,
        ):
            if n_masks:
                mk = mk_pool.tile([128, n_masks, QM], bf16)
                nc.sync.dma_start(mk[:], mk_ext[:])
            if use_pad:
                pc = mk_pool.tile([128, NKB], bf16)
                nc.sync.dma_start(pc[:], pc_ext[:])

            for h in range(HPC):
                qt = qt_pool.tile([128, S], bf16)
                nc.sync.dma_start(qt[:], qt_ext[h])
                kt = kt_pool.tile([128, S], bf16)
                nc.sync.dma_start(kt[:], kt_ext[h])
                vp = vp_pool.tile([128, NKB, VW], bf16)
                nc.sync.dma_start(vp[:], vp_ext[h])

                for m in range(NM):
                    blocks = schedule[m]
                    if not blocks:
                        continue
                    # one open PSUM accumulation group per bank — the two q
                    # sub-outputs need separate banks
                    o_subs = [
                        o_pool.tile([128, VW], f32, tag="o", name=f"o{s_}")
                        for s_ in (0, 1)
                    ]
                    n_blk = len(blocks)
                    groups = [blocks[i : i + G] for i in range(0, n_blk, G)]
                    gi = 0
                    for grp in groups:
                        ng = len(grp)
                        st = st_pool.tile([128, G * QM], f32, tag="st")
                        for gidx, (j, mi) in enumerate(grp):
                            nc.tensor.matmul(
                                st[:, gidx * QM : (gidx + 1) * QM],
                                lhsT=kt[:, j * KB : (j + 1) * KB],
                                rhs=qt[:, m * QM : (m + 1) * QM],
                                start=True,
                                stop=True,
                            )
                        pt = pt_pool.tile([128, G * QM], bf16, tag="pt")
                        nc.scalar.activation(
                            pt[:, : ng * QM], st[:, : ng * QM], Exp, scale=SCALE
                        )
                        for gidx, (j, mi) in enumerate(grp):
                            psl = pt[:, gidx * QM : (gidx + 1) * QM]
                            if mi is not None:
                                nc.gpsimd.tensor_mul(psl, psl, mk[:, mi, :])
                            if use_pad:
                                nc.gpsimd.tensor_scalar_mul(psl, psl, pc[:, j : j + 1])
                            idx = gi + gidx
                            first = idx == 0
                            last = idx == n_blk - 1
                            for sub in (0, 1):
                                nc.tensor.matmul(
                                    o_subs[sub][:],
                                    lhsT=pt[:, gidx * QM + sub * 128 : gidx * QM + sub * 128 + 128],
                                    rhs=vp[:, j, :],
                                    start=first,
                                    stop=last,
                                )
                        gi += ng
                    for sub in (0, 1):
                        o = o_subs[sub]
                        rec = rec_pool.tile([128, 1], f32)
                        nc.vector.reciprocal(rec[:], o[:, D : D + 1])
                        osb = osb_pool.tile([128, D], f32)
                        nc.vector.tensor_scalar_mul(osb[:], o[:, 0:D], rec[:])
                        row0 = m * QM + sub * 128
                        nc.sync.dma_start(out_ext[h, row0 : row0 + 128, :], osb[:])
    nc.compile()
    return nc


def _prep_inputs(q, k, v, attn_mask, pad_mask):
    q = np.asarray(q, dtype=np.float32).reshape(BH, S, D)
    k = np.asarray(k, dtype=np.float32).reshape(BH, S, D)
    v = np.asarray(v, dtype=np.float32).reshape(BH, S, D)

    qt = np.ascontiguousarray(q.transpose(0, 2, 1)).astype(ml_dtypes.bfloat16)
    kt = np.ascontiguousarray(k.transpose(0, 2, 1)).astype(ml_dtypes.bfloat16)

    # V': [BH, 128(row within k block), NKB, VW] bf16; col D = 1.0 (denominator)
    vp = np.zeros((BH, 128, NKB, VW), dtype=ml_dtypes.bfloat16)
    vblocks = v.reshape(BH, NKB, 128, D).transpose(0, 2, 1, 3)
    vp[:, :, :, :D] = vblocks.astype(ml_dtypes.bfloat16)
    vp[:, :, :, D] = 1.0

    pad = np.asarray(pad_mask).reshape(B, S) != 0
    use_pad = not bool(pad.all())
    pcs = None
    if use_pad:
        pcs = []
        for c in range(NCORES):
            b = (c * HPC) // H
            pcs.append(
                np.ascontiguousarray(
                    pad[b].reshape(NKB, 128).T.astype(ml_dtypes.bfloat16)
                )
            )
    return qt, kt, vp, use_pad, pcs


def kernel(q, k, v, attn_mask, pad_mask):
    global LAST_RESULTS
    from concourse.bass_utils import run_bass_kernel_spmd

    schedule, mask_tiles = _derive_schedule(attn_mask)
    qt, kt, vp, use_pad, pcs = _prep_inputs(q, k, v, attn_mask, pad_mask)
    n_masks = 0 if mask_tiles is None else mask_tiles.shape[1]

    key = (
        np.asarray(attn_mask).tobytes(),
        use_pad,
    )
    nc = _CACHE.get(key)
    if nc is None:
        nc = _build_program(schedule, n_masks, use_pad)
        _CACHE[key] = nc

    in_maps = []
    for c in range(NCORES):
        sl = slice(c * HPC, (c + 1) * HPC)
        m = {"qt": qt[sl], "kt": kt[sl], "vp": vp[sl]}
        if n_masks:
            m["mk"] = mask_tiles
        if use_pad:
            m["pc"] = pcs[c]
        in_maps.append(m)

    res = run_bass_kernel_spmd(nc, in_maps, core_ids=list(range(NCORES)))
    LAST_RESULTS = res
    out = np.concatenate([res.results[c]["out"] for c in range(NCORES)], axis=0)
    return np.ascontiguousarray(out.reshape(B, H, S, D).astype(np.float32))


# revision 9
# speedup vs baseline: 1.2136x; 1.1904x over previous
"""Multi-head attention (B=4, H=16, S=2048, D=128, causal+pad mask) on 8 TRN2 NeuronCores.

Sharding: the 64 (batch, head) pairs are split 8 per core (pure data parallel —
attention is independent per head, no collectives needed).

Per-core kernel (per head):
  - scores are computed TRANSPOSED: S^T[k, q] = (K_block)^T^T @ Q^T with the
    contraction dim d=128 on partitions, k-block (128) as the PSUM partition dim
    and a 256-wide q tile as the moving dim. Inputs are fp32 bitcast to float32r,
    which runs at full PE rate (1 cycle/row) for moving dim >= 256.
  - exp(scale*s) runs on the scalar engine straight out of PSUM into SBUF as
    bf16 (P^T layout). No max-subtraction: scores*scale ~ N(0,1), exp is safe.
  - Partially-masked blocks are zeroed by a bf16 multiply with host-derived mask
    tiles (deduped). Fully-masked blocks are skipped; fully-allowed untouched.
  - P^T lands exactly in the layout the PV matmul needs (k on partitions):
    O[q_sub 128, 132] += P^T[:, sub]^T @ V'[k_block] accumulated over k blocks
    in PSUM, where V' is V in bf16 with a ones column appended at col 128 —
    so O[:, 128] is the softmax denominator for free.
  - reciprocal + per-partition scale normalizes, then DMA out as f32.
"""

import os
import sys

import numpy as np

try:  # the repo root that provides `concourse` / `gauge` / `antenv`
    import concourse.bass  # noqa: F401
except ImportError:  # pragma: no cover
    for _p in ("/opt/trn_rl_repo", "/root/.axon_site/_ro/trn_rl_repo"):
        if os.path.isdir(_p) and _p not in sys.path:
            sys.path.insert(0, _p)

import ml_dtypes

B, H, S, D = 4, 16, 2048, 128
BH = B * H
NCORES = 8
HPC = BH // NCORES  # heads per core = 8
QM = 256  # q tile width (matmul moving dim; >=256 keeps float32r at full rate)
KB = 128  # k block (PSUM partition dim of S^T)
NM = S // QM  # 8 q tiles
NKB = S // KB  # 16 k blocks
VW = D + 4  # V' width: col D holds ones (softmax denom), cols D+1.. are zero pad
SCALE = float(np.float32(1.0 / np.sqrt(np.float32(D))))

_CACHE: dict = {}
LAST_RESULTS = None  # BassKernelResults of the most recent run (for test harness)


def _derive_schedule(attn_mask):
    """Classify each (q-tile, k-block) pair from the actual mask.

    Returns (schedule, mask_tiles) where schedule[m] is a list of
    (j, mask_idx_or_None) for blocks with any allowed element, and mask_tiles is
    the deduped [128, n, QM] bf16 stack of transposed partial-block masks.
    """
    am = np.asarray(attn_mask) != 0  # [S(q), S(k)] bool
    uniq: dict = {}
    tiles = []
    schedule = []
    for m in range(NM):
        row = []
        for j in range(NKB):
            blk = am[m * QM : (m + 1) * QM, j * KB : (j + 1) * KB]  # [QM, KB]
            if not blk.any():
                continue
            if blk.all():
                row.append((j, None))
                continue
            key = blk.tobytes()
            if key not in uniq:
                uniq[key] = len(tiles)
                tiles.append(blk.T.astype(ml_dtypes.bfloat16))  # [KB, QM]
            row.append((j, uniq[key]))
        schedule.append(row)
    mask_tiles = np.stack(tiles, axis=1) if tiles else None  # [128, n, QM]
    return schedule, mask_tiles


def _build_program(schedule, n_masks, use_pad):
    import concourse.mybir as mybir
    import concourse.tile as tile
    from concourse import bacc

    f32 = mybir.dt.float32
    f32r = mybir.dt.float32r
    bf16 = mybir.dt.bfloat16
    Exp = mybir.ActivationFunctionType.Exp

    nc = bacc.Bacc(None)
    qt_ext = nc.declare_dram_parameter("qt", [HPC, 128, S], bf16, isOutput=False)
    kt_ext = nc.declare_dram_parameter("kt", [HPC, 128, S], bf16, isOutput=False)
    vp_ext = nc.declare_dram_parameter("vp", [HPC, 128, NKB, VW], bf16, isOutput=False)
    if n_masks:
        mk_ext = nc.declare_dram_parameter("mk", [128, n_masks, QM], bf16, isOutput=False)
    if use_pad:
        pc_ext = nc.declare_dram_parameter("pc", [128, NKB], bf16, isOutput=False)
    out_ext = nc.declare_dram_parameter("out", [HPC, S, D], f32, isOutput=True)

    G = 4  # k-blocks per exp group: st group [128, G*QM] f32 = 2 PSUM banks

    with tile.TileContext(nc) as tc:
        with (
            tc.tile_pool(name="qt", bufs=2) as qt_pool,
            tc.tile_pool(name="kt", bufs=2) as kt_pool,
            tc.tile_pool(name="vp", bufs=2) as vp_pool,
            tc.tile_pool(name="pt", bufs=3) as pt_pool,
            tc.tile_pool(name="osb", bufs=3) as osb_pool,
            tc.tile_pool(name="rec", bufs=3) as rec_pool,
            tc.tile_pool(name="mk", bufs=1) as mk_pool,
            tc.tile_pool(name="st", bufs=2, space="PSUM") as st_pool,
            tc.tile_pool(name="ops", bufs=4, space="PSUM") as o_pool,
        ):
            if n_masks:
                mk = mk_pool.tile([128, n_masks, QM], bf16)
                nc.sync.dma_start(mk[:], mk_ext[:])
            if use_pad:
                pc = mk_pool.tile([128, NKB], bf16)
                nc.sync.dma_start(pc[:], pc_ext[:])

            for h in range(HPC):
                qt = qt_pool.tile([128, S], bf16)
                nc.sync.dma_start(qt[:], qt_ext[h])
                kt = kt_pool.tile([128, S], bf16)
                nc.sync.dma_start(kt[:], kt_ext[h])
                vp = vp_pool.tile([128, NKB, VW], bf16)
                nc.sync.dma_start(vp[:], vp_ext[h])

                for m in range(NM):
                    blocks = schedule[m]
                    if not blocks:
                        continue
                    # one open PSUM accumulation group per bank — the two q
                    # sub-outputs need separate banks
                    o_subs = [
                        o_pool.tile([128, VW], f32, tag="o", name=f"o{s_}")
                        for s_ in (0, 1)
                    ]
                    n_blk = len(blocks)
                    groups = [blocks[i : i + G] for i in range(0, n_blk, G)]
                    gi = 0
                    for grp in groups:
                        ng = len(grp)
                        st = st_pool.tile([128, G * QM], f32, tag="st")
                        for gidx, (j, mi) in enumerate(grp):
                            nc.tensor.matmul(
                                st[:, gidx * QM : (gidx + 1) * QM],
                                lhsT=kt[:, j * KB : (j + 1) * KB],
                                rhs=qt[:, m * QM : (m + 1) * QM],
                                start=True,
                                stop=True,
                            )
                        pt = pt_pool.tile([128, G * QM], bf16, tag="pt")
                        nc.scalar.activation(
                            pt[:, : ng * QM], st[:, : ng * QM], Exp, scale=SCALE
                        )
                        for gidx, (j, mi) in enumerate(grp):
                            psl = pt[:, gidx * QM : (gidx + 1) * QM]
                            if mi is not None:
                                nc.vector.tensor_mul(psl, psl, mk[:, mi, :])
                            if use_pad:
                                nc.vector.tensor_scalar_mul(psl, psl, pc[:, j : j + 1])
                            idx = gi + gidx
                            first = idx == 0
                            last = idx == n_blk - 1
                            for sub in (0, 1):
                                nc.tensor.matmul(
                                    o_subs[sub][:],
                                    lhsT=pt[:, gidx * QM + sub * 128 : gidx * QM + sub * 128 + 128],
                                    rhs=vp[:, j, :],
                                    start=first,
                                    stop=last,
                                )
                        gi += ng
                    for sub in (0, 1):
                        o = o_subs[sub]
                        rec = rec_pool.tile([128, 1], f32)
                        nc.vector.reciprocal(rec[:], o[:, D : D + 1])
                        osb = osb_pool.tile([128, D], f32)
                        nc.vector.tensor_scalar_mul(osb[:], o[:, 0:D], rec[:])
                        row0 = m * QM + sub * 128
                        nc.sync.dma_start(out_ext[h, row0 : row0 + 128, :], osb[:])
    nc.compile()
    return nc


def _prep_inputs(q, k, v, attn_mask, pad_mask):
    q = np.asarray(q, dtype=np.float32).reshape(BH, S, D)
    k = np.asarray(k, dtype=np.float32).reshape(BH, S, D)
    v = np.asarray(v, dtype=np.float32).reshape(BH, S, D)

    qt = np.ascontiguousarray(q.transpose(0, 2, 1)).astype(ml_dtypes.bfloat16)
    kt = np.ascontiguousarray(k.transpose(0, 2, 1)).astype(ml_dtypes.bfloat16)

    # V': [BH, 128(row within k block), NKB, VW] bf16; col D = 1.0 (denominator)
    vp = np.zeros((BH, 128, NKB, VW), dtype=ml_dtypes.bfloat16)
    vblocks = v.reshape(BH, NKB, 128, D).transpose(0, 2, 1, 3)
    vp[:, :, :, :D] = vblocks.astype(ml_dtypes.bfloat16)
    vp[:, :, :, D] = 1.0

    pad = np.asarray(pad_mask).reshape(B, S) != 0
    use_pad = not bool(pad.all())
    pcs = None
    if use_pad:
        pcs = []
        for c in range(NCORES):
            b = (c * HPC) // H
            pcs.append(
                np.ascontiguousarray(
                    pad[b].reshape(NKB, 128).T.astype(ml_dtypes.bfloat16)
                )
            )
    return qt, kt, vp, use_pad, pcs


def kernel(q, k, v, attn_mask, pad_mask):
    global LAST_RESULTS
    from concourse.bass_utils import run_bass_kernel_spmd

    schedule, mask_tiles = _derive_schedule(attn_mask)
    qt, kt, vp, use_pad, pcs = _prep_inputs(q, k, v, attn_mask, pad_mask)
    n_masks = 0 if mask_tiles is None else mask_tiles.shape[1]

    key = (
        np.asarray(attn_mask).tobytes(),
        use_pad,
    )
    nc = _CACHE.get(key)
    if nc is None:
        nc = _build_program(schedule, n_masks, use_pad)
        _CACHE[key] = nc

    in_maps = []
    for c in range(NCORES):
        sl = slice(c * HPC, (c + 1) * HPC)
        m = {"qt": qt[sl], "kt": kt[sl], "vp": vp[sl]}
        if n_masks:
            m["mk"] = mask_tiles
        if use_pad:
            m["pc"] = pcs[c]
        in_maps.append(m)

    res = run_bass_kernel_spmd(nc, in_maps, core_ids=list(range(NCORES)))
    LAST_RESULTS = res
    out = np.concatenate([res.results[c]["out"] for c in range(NCORES)], axis=0)
    return np.ascontiguousarray(out.reshape(B, H, S, D).astype(np.float32))


# revision 11
# speedup vs baseline: 1.3026x; 1.0733x over previous
"""Multi-head attention (B=4, H=16, S=2048, D=128, causal+pad mask) on 8 TRN2 NeuronCores.

Sharding: the 64 (batch, head) pairs are split 8 per core (pure data parallel —
attention is independent per head, no collectives needed).

Per-core kernel (per head):
  - scores are computed TRANSPOSED: S^T[k, q] = K_block^T^T @ Q^T with the
    contraction dim d=128 on partitions, the k-block (128) as the PSUM partition
    dim and the allowed q-column range of the block (128-col granular, derived
    from the actual mask) as the moving dim. Q/K are host-cast to bf16.
  - Consecutive score segments are packed into [128, <=1024] PSUM group tiles
    (2 banks) so ONE scalar-engine ACTIVATE computes exp(scale*s) for the whole
    group straight out of PSUM into SBUF bf16 — amortizing the ~352-cycle ACT
    pipeline fill. No max-subtraction: scores*scale ~ N(0,1), exp is safe.
  - Partially-masked 128x128 chunks are zeroed by a bf16 multiply with
    host-derived deduped mask tiles on the vector engine. Fully-masked chunks
    are never computed; fully-allowed chunks are untouched.
  - P^T lands exactly in the layout the PV matmul needs (k on partitions):
    O[q_sub 128, 132] += P^T[:, chunk]^T @ V'[k_block] accumulated over k
    blocks in PSUM, where V' is V in bf16 with a ones column appended at col
    128 — so O[:, 128] is the softmax denominator for free.
  - reciprocal + per-partition scale normalizes, then DMA out as f32.
"""

import os
import sys
from collections import defaultdict

import numpy as np

try:  # the repo root that provides `concourse` / `gauge`
    import concourse.bass  # noqa: F401
except ImportError:  # pragma: no cover
    for _p in ("/opt/trn_rl_repo", "/root/.axon_site/_ro/trn_rl_repo"):
        if os.path.isdir(_p) and _p not in sys.path:
            sys.path.insert(0, _p)

import ml_dtypes

B, H, S, D = 4, 16, 2048, 128
BH = B * H
NCORES = 8
HPC = BH // NCORES  # heads per core = 8
QM = 256  # q megatile width; q sub-chunks of 128 map to PV output tiles
CH = 128  # q chunk granularity (PV stationary width / mask tile width)
KB = 128  # k block (PSUM partition dim of S^T)
NM = S // QM  # 8 q megatiles
NKB = S // KB  # 16 k blocks
VW = D + 4  # V' width: col D holds ones (softmax denom), cols D+1.. are zero pad
GCOLS = 1024  # exp group columns: [128, 1024] f32 = 2 PSUM banks
SCALE = float(np.float32(1.0 / np.sqrt(np.float32(D))))
NSUB = QM // CH  # q sub-chunks per megatile = 2

_CACHE: dict = {}
LAST_RESULTS = None  # BassKernelResults of the most recent run (for test harness)


def _derive_schedule(attn_mask):
    """Derive the packed block schedule from the actual mask.

    Returns (stream, contrib, mask_tiles):
      stream: ordered [(m, j, lo, hi, cmasks)] score segments; [lo, hi) is the
        128-granular allowed q-col range of block (m, j); cmasks lists
        (chunk_offset_within_seg, mask_id) for partially-masked 128-chunks.
      contrib: {(m, sub): n} count of PV contributions per output sub-tile.
      mask_tiles: [128, n_masks, CH] bf16 deduped transposed chunk masks.
    """
    am = np.asarray(attn_mask) != 0  # [S(q), S(k)]
    uniq: dict = {}
    tiles = []
    stream = []
    contrib: dict = defaultdict(int)
    for m in range(NM):
        for j in range(NKB):
            chunks = []
            for c in range(NSUB):
                cm = am[m * QM + c * CH : m * QM + (c + 1) * CH, j * KB : (j + 1) * KB]
                if not cm.any():
                    chunks.append(None)
                elif cm.all():
                    chunks.append("f")
                else:
                    key = cm.tobytes()
                    if key not in uniq:
                        uniq[key] = len(tiles)
                        tiles.append(cm.T.astype(ml_dtypes.bfloat16))  # [KB, CH]
                    chunks.append(uniq[key])
            c = 0
            while c < NSUB:
                if chunks[c] is None:
                    c += 1
                    continue
                c0 = c
                while c < NSUB and chunks[c] is not None:
                    c += 1
                cmasks = [
                    (cc - c0, chunks[cc]) for cc in range(c0, c) if chunks[cc] != "f"
                ]
                stream.append((m, j, c0 * CH, c * CH, cmasks))
                for cc in range(c0, c):
                    contrib[(m, cc)] += 1
    mask_tiles = np.stack(tiles, axis=1) if tiles else None  # [128, n, CH]
    return stream, dict(contrib), mask_tiles


def _build_program(stream, contrib, n_masks, use_pad):
    import concourse.mybir as mybir
    import concourse.tile as tile
    from concourse import bacc

    f32 = mybir.dt.float32
    bf16 = mybir.dt.bfloat16
    Exp = mybir.ActivationFunctionType.Exp

    nc = bacc.Bacc(None)
    qt_ext = nc.declare_dram_parameter("qt", [HPC, 128, S], bf16, isOutput=False)
    kt_ext = nc.declare_dram_parameter("kt", [HPC, 128, S], bf16, isOutput=False)
    vp_ext = nc.declare_dram_parameter("vp", [HPC, 128, NKB, VW], bf16, isOutput=False)
    if n_masks:
        mk_ext = nc.declare_dram_parameter("mk", [128, n_masks, CH], bf16, isOutput=False)
    if use_pad:
        pc_ext = nc.declare_dram_parameter("pc", [128, NKB], bf16, isOutput=False)
    out_ext = nc.declare_dram_parameter("out", [HPC, S, D], f32, isOutput=True)

    # pack the segment stream into exp groups of <= GCOLS columns
    groups = []
    cur, cols = [], 0
    for seg in stream:
        w = seg[3] - seg[2]
        if cols + w > GCOLS:
            groups.append((cur, cols))
            cur, cols = [], 0
        cur.append(seg)
        cols += w
    if cur:
        groups.append((cur, cols))

    with tile.TileContext(nc) as tc:
        with (
            tc.tile_pool(name="qt", bufs=2) as qt_pool,
            tc.tile_pool(name="kt", bufs=2) as kt_pool,
            tc.tile_pool(name="vp", bufs=2) as vp_pool,
            tc.tile_pool(name="pt", bufs=3) as pt_pool,
            tc.tile_pool(name="osb", bufs=4) as osb_pool,
            tc.tile_pool(name="rec", bufs=4) as rec_pool,
            tc.tile_pool(name="mk", bufs=1) as mk_pool,
            tc.tile_pool(name="st", bufs=2, space="PSUM") as st_pool,
            tc.tile_pool(name="ops", bufs=4, space="PSUM") as o_pool,
        ):
            if n_masks:
                mk = mk_pool.tile([128, n_masks, CH], bf16)
                nc.sync.dma_start(mk[:], mk_ext[:])
            if use_pad:
                pc = mk_pool.tile([128, NKB], bf16)
                nc.sync.dma_start(pc[:], pc_ext[:])

            NQ = 4  # input DMA quarters — lets compute start before full tiles land
            for h in range(HPC):
                qt = qt_pool.tile([128, S], bf16)
                kt = kt_pool.tile([128, S], bf16)
                vp = vp_pool.tile([128, NKB, VW], bf16)
                qs = S // NQ
                js = NKB // NQ
                for q4 in range(NQ):
                    nc.sync.dma_start(
                        kt[:, q4 * qs : (q4 + 1) * qs], kt_ext[h, :, q4 * qs : (q4 + 1) * qs]
                    )
                    nc.sync.dma_start(
                        qt[:, q4 * qs : (q4 + 1) * qs], qt_ext[h, :, q4 * qs : (q4 + 1) * qs]
                    )
                    nc.sync.dma_start(
                        vp[:, q4 * js : (q4 + 1) * js, :],
                        vp_ext[h, :, q4 * js : (q4 + 1) * js, :],
                    )

                o_tiles: dict = {}
                seen: dict = defaultdict(int)

                def finalize(m, sub, o):
                    rec = rec_pool.tile([128, 1], f32, name="rec")
                    nc.vector.reciprocal(rec[:], o[:, D : D + 1])
                    osb = osb_pool.tile([128, D], f32, name="osb")
                    nc.vector.tensor_scalar_mul(osb[:], o[:, 0:D], rec[:])
                    row0 = m * QM + sub * CH
                    nc.sync.dma_start(out_ext[h, row0 : row0 + CH, :], osb[:])

                for grp, gcols in groups:
                    st = st_pool.tile([128, GCOLS], f32, tag="st", name="st")
                    p = 0
                    for m, j, lo, hi, cmasks in grp:
                        w = hi - lo
                        off = 0
                        while off < w:  # matmul output must not cross a PSUM bank
                            wseg = min(w - off, 512 - (p + off) % 512)
                            nc.tensor.matmul(
                                st[:, p + off : p + off + wseg],
                                lhsT=kt[:, j * KB : (j + 1) * KB],
                                rhs=qt[:, m * QM + lo + off : m * QM + lo + off + wseg],
                                start=True,
                                stop=True,
                            )
                            off += wseg
                        p += w
                    pt = pt_pool.tile([128, GCOLS], bf16, tag="pt", name="pt")
                    nc.scalar.activation(pt[:, :gcols], st[:, :gcols], Exp, scale=SCALE)
                    p = 0
                    for m, j, lo, hi, cmasks in grp:
                        w = hi - lo
                        for coff, mi in cmasks:
                            nc.vector.tensor_mul(
                                pt[:, p + coff * CH : p + (coff + 1) * CH],
                                pt[:, p + coff * CH : p + (coff + 1) * CH],
                                mk[:, mi, :],
                            )
                        if use_pad:
                            nc.vector.tensor_scalar_mul(
                                pt[:, p : p + w], pt[:, p : p + w], pc[:, j : j + 1]
                            )
                        if m not in o_tiles:
                            o_tiles[m] = [
                                o_pool.tile([128, VW], f32, tag="o", name=f"o{s_}")
                                for s_ in range(NSUB)
                            ]
                        for c in range(w // CH):
                            sub = lo // CH + c
                            key = (m, sub)
                            seen[key] += 1
                            nc.tensor.matmul(
                                o_tiles[m][sub][:],
                                lhsT=pt[:, p + c * CH : p + (c + 1) * CH],
                                rhs=vp[:, j, :],
                                start=seen[key] == 1,
                                stop=seen[key] == contrib[key],
                            )
                            if seen[key] == contrib[key]:
                                finalize(m, sub, o_tiles[m][sub])
                        p += w
    nc.compile()
    return nc


def _prep_inputs(q, k, v, attn_mask, pad_mask):
    q = np.asarray(q, dtype=np.float32).reshape(BH, S, D)
    k = np.asarray(k, dtype=np.float32).reshape(BH, S, D)
    v = np.asarray(v, dtype=np.float32).reshape(BH, S, D)

    qt = np.ascontiguousarray(q.transpose(0, 2, 1)).astype(ml_dtypes.bfloat16)
    kt = np.ascontiguousarray(k.transpose(0, 2, 1)).astype(ml_dtypes.bfloat16)

    # V': [BH, 128(row within k block), NKB, VW] bf16; col D = 1.0 (denominator)
    vp = np.zeros((BH, 128, NKB, VW), dtype=ml_dtypes.bfloat16)
    vblocks = v.reshape(BH, NKB, 128, D).transpose(0, 2, 1, 3)
    vp[:, :, :, :D] = vblocks.astype(ml_dtypes.bfloat16)
    vp[:, :, :, D] = 1.0

    pad = np.asarray(pad_mask).reshape(B, S) != 0
    use_pad = not bool(pad.all())
    pcs = None
    if use_pad:
        pcs = []
        for c in range(NCORES):
            b = (c * HPC) // H
            pcs.append(
                np.ascontiguousarray(
                    pad[b].reshape(NKB, 128).T.astype(ml_dtypes.bfloat16)
                )
            )
    return qt, kt, vp, use_pad, pcs


def kernel(q, k, v, attn_mask, pad_mask):
    global LAST_RESULTS
    from concourse.bass_utils import run_bass_kernel_spmd

    stream, contrib, mask_tiles = _derive_schedule(attn_mask)
    qt, kt, vp, use_pad, pcs = _prep_inputs(q, k, v, attn_mask, pad_mask)
    n_masks = 0 if mask_tiles is None else mask_tiles.shape[1]

    key = (np.asarray(attn_mask).tobytes(), use_pad)
    nc = _CACHE.get(key)
    if nc is None:
        nc = _build_program(stream, contrib, n_masks, use_pad)
        _CACHE[key] = nc

    in_maps = []
    for c in range(NCORES):
        sl = slice(c * HPC, (c + 1) * HPC)
        m = {"qt": qt[sl], "kt": kt[sl], "vp": vp[sl]}
        if n_masks:
            m["mk"] = mask_tiles
        if use_pad:
            m["pc"] = pcs[c]
        in_maps.append(m)

    res = run_bass_kernel_spmd(nc, in_maps, core_ids=list(range(NCORES)))
    LAST_RESULTS = res
    out = np.concatenate([res.results[c]["out"] for c in range(NCORES)], axis=0)
    return np.ascontiguousarray(out.reshape(B, H, S, D).astype(np.float32))
